# revision 18
# baseline (speedup 1.0000x reference)
"""Trainium2 Bass kernel for nn_Attention_72670846649042.

GRU encoder + greedy attention decoder, B=512,L=25,H=1024,D=256,T=128,E=300.
Sharding: data-parallel over batch, 64 rows/core on 8 cores, no collectives.

v3 design:
 - Host precomputes the encoder input projection gi (bf16) and all layout
   transforms; device runs only the two recurrences.
 - Partition-packed elementwise: gate PSUMs are (128, 512) holding both
   512-column halves of the hidden dim on partition ranges [0:64)/[64:128)
   (matmul quadrant tile_position), so every gate op runs at full DVE width.
 - GRU state is a single persistent bf16 (128,512) tile; h2 = zh + (1-z)*n
   with zh/(1-z) precomputed off the critical chain.
 - hT tiles rebuilt via PE transposes (identity matmul); rolling 4-slot hT
   history feeds the pairwise EW precompute (EW = enc_out @ comb_W2.T)
   interleaved into the encoder; per decoder step attention-apply + comb
   collapse into one PSUM-accumulated block-diag matmul over EW.
 - All softmaxes use exp(x)=(1+tanh(x/2))/(1-tanh(x/2)) with max subtraction;
   log-softmax denominators deferred to one Ln at the end. The whole loop
   uses one activation table (sigmoid/tanh/relu/copy).
 - Biases in this instance are all zero; bias matmuls emitted only if nonzero.
"""
import os
import numpy as np
import ml_dtypes

B, L, V, E, H, D, T = 512, 25, 50000, 300, 1024, 256, 128
NC = 8
BL = B // NC          # 64 local batch
G3 = 3 * H            # 3072
KH = H // 128         # 8 hidden ktiles
KC = (D + H) // 128   # 10 ktiles for concat(emb, h/applied)
NP = 13               # l-pairs (2 l per 128-row K tile); l=25 is zero-padded
MAXN1, MAXN2, BN_EPS = 10.0, 1.0, 1e-5
BF16 = ml_dtypes.bfloat16

LINEARIZE = False


def build_nc(s2_scale, biases):
    """biases: dict name -> np row [1,X] or None (zero => op not emitted)."""
    import concourse.bass as bass
    import concourse.tile as tile
    from concourse import bacc, mybir
    from contextlib import ExitStack

    dt = mybir.dt
    AF = mybir.ActivationFunctionType
    ALU = mybir.AluOpType
    AX = mybir.AxisListType

    nc = bacc.Bacc("TRN2", target_bir_lowering=False, debug=False)

    # ---- dram parameters ----
    # gi rows: t*128 + hc*64 + b, cols [r|z|n] (512 each) for that hc
    gi_d = nc.declare_dram_parameter("gi", [L * 2 * BL, 1536], dt.bfloat16, isOutput=False)
    encWhhT_d = nc.declare_dram_parameter("encWhhT", [H, G3], dt.bfloat16, isOutput=False)
    decWihT_d = nc.declare_dram_parameter("decWihT", [H, G3], dt.bfloat16, isOutput=False)
    decWhhT_d = nc.declare_dram_parameter("decWhhT", [H, G3], dt.bfloat16, isOutput=False)
    combWT_d = nc.declare_dram_parameter("combWT", [D + H, H], dt.bfloat16, isOutput=False)
    outWTs_d = nc.declare_dram_parameter("outWTs", [H, T], dt.bfloat16, isOutput=False)
    attnWT_d = nc.declare_dram_parameter("attnWT", [D + H, L], dt.bfloat16, isOutput=False)
    embbf_d = nc.declare_dram_parameter("embbf", [128, D], dt.bfloat16, isOutput=False)
    sosT_d = nc.declare_dram_parameter("sosT", [128, 2 * BL], dt.bfloat16, isOutput=False)
    istk_d = nc.declare_dram_parameter("istk", [128, BL], dt.bfloat16, isOutput=False)
    bias_d = {}
    for k, v in biases.items():
        if v is not None:
            bias_d[k] = nc.declare_dram_parameter(k, list(v.shape), dt.bfloat16,
                                                  isOutput=False)
    out_d = nc.declare_dram_parameter("out", [BL * L, T], dt.float32, isOutput=True)

    with tile.TileContext(nc, linearize=LINEARIZE) as tc, ExitStack() as ctx:
        # ================= static pools =================
        shared = ctx.enter_context(tc.tile_pool(name="shared", bufs=1))
        work = ctx.enter_context(tc.tile_pool(name="work", bufs=2))
        small = ctx.enter_context(tc.tile_pool(name="small", bufs=2))

        decWhhT = shared.tile([128, KH, G3], dt.bfloat16, tag="decWhhT")
        EW = shared.tile([128, NP, H], dt.bfloat16, tag="EW")
        hst = shared.tile([128, 512], dt.bfloat16, tag="hst")  # h state, packed
        Istk = shared.tile([128, BL], dt.bfloat16, tag="Istk")
        combW01 = shared.tile([128, 2, H], dt.bfloat16, tag="combW01")
        hTab = [shared.tile([128, KH, BL], dt.bfloat16, tag=f"hT{i}", name=f"hT{i}")
                for i in range(2)]
        outWTs = shared.tile([128, KH, T], dt.bfloat16, tag="outWTs")
        attnWT = shared.tile([128, KC, L], dt.bfloat16, tag="attnWT")
        embbf = shared.tile([128, D], dt.bfloat16, tag="embbf")
        embT = shared.tile([128, 2, BL], dt.bfloat16, tag="embT")
        bias_t = {}
        for k in bias_d:
            bias_t[k] = shared.tile(list(biases[k].shape), dt.bfloat16, tag=k, name=k)
            nc.sync.dma_start(bias_t[k][:], bias_d[k].ap())
        if bias_d:
            ones_sb = shared.tile([1, BL], dt.bfloat16, tag="ones_sb")
            nc.vector.memset(ones_sb[:], 1.0)

        # small DMAs first so they don't queue behind the big weights
        nc.sync.dma_start(Istk[:], istk_d.ap())
        nc.sync.dma_start(embT[:], sosT_d.ap().rearrange("p (k b) -> p k b", b=BL))
        nc.sync.dma_start(embbf[:], embbf_d.ap())
        nc.sync.dma_start(attnWT[:], attnWT_d.ap().rearrange("(k p) n -> p k n", p=128))
        nc.sync.dma_start(outWTs[:], outWTs_d.ap().rearrange("(k p) n -> p k n", p=128))
        nc.vector.memset(hst[:], 0.0)

        def ksl(w, k, lo, n=512):  # weight tile slice helper
            return w[:, k, lo:lo + n]

        def transposes(tpt, src, stg, dstv, dsts):
            """PE-transpose packed (128,512) bf16 src into [128, k, 64] dst.
            The PE cannot switch lhsT partition base between ops, so the upper
            half is staged to partitions 0-63 via SBUF-SBUF DMA first."""
            for q in range(4):
                eng = nc.sync if q % 2 == 0 else nc.scalar
                eng.dma_start(stg[:, q * 128:(q + 1) * 128],
                              src[64:128, q * 128:(q + 1) * 128])
            for k in range(4):
                nc.tensor.transpose(tpt[:, k, :], src[0:64, k * 128:(k + 1) * 128],
                                    Istk[0:BL, :])
            nc.vector.tensor_copy(dstv, tpt[:, 0:4, :])
            for k in range(4, KH):
                nc.tensor.transpose(tpt[:, k, :], stg[:, (k - 4) * 128:(k - 3) * 128],
                                    Istk[0:BL, :])
            nc.scalar.copy(dsts, tpt[:, 4:8, :])

        # ================= encoder phase =================
        with tc.tile_pool(name="encw", bufs=1) as encw, \
             tc.tile_pool(name="gip", bufs=3) as gip, \
             tc.tile_pool(name="encps", bufs=2, space="PSUM") as encps, \
             tc.tile_pool(name="tpps", bufs=1, space="PSUM") as tpps, \
             tc.tile_pool(name="ewps", bufs=1, space="PSUM") as ewps:
            encWhhT = encw.tile([128, KH, G3], dt.bfloat16, tag="encWhhT")
            for k in range(KH):
                for h4 in range(4):
                    nc.sync.dma_start(encWhhT[:, k, h4 * 768:(h4 + 1) * 768],
                                      encWhhT_d.ap()[k * 128:(k + 1) * 128,
                                                     h4 * 768:(h4 + 1) * 768])
            combWT = encw.tile([128, KC, H], dt.bfloat16, tag="combWT")
            cwr = combWT_d.ap().rearrange("(k p) n -> p k n", p=128)
            for k in range(KC):
                nc.sync.dma_start(combWT[:, k, :], cwr[:, k, :])
            dwr = decWhhT_d.ap().rearrange("(k p) n -> p k n", p=128)
            for k in range(KH):
                nc.sync.dma_start(decWhhT[:, k, :], dwr[:, k, :])

            # rolling hT history: slot t%4 holds h2T(t); slot 3 = h(-1) = 0
            hTr = encw.tile([128, KH, 4 * BL], dt.bfloat16, tag="hTr")
            nc.vector.memset(hTr[:, :, 3 * BL:4 * BL], 0.0)

            def ew_half(p, n2):
                lo = (2 * p) % 4 * BL
                ewp = ewps.tile([128, 512], dt.float32, tag="ewp")
                for k in range(KH):
                    nc.tensor.matmul(ewp[:], hTr[:, k, lo:lo + 128],
                                     ksl(combWT, 2 + k, n2 * 512),
                                     start=(k == 0), stop=(k == KH - 1))
                if n2 == 0:
                    nc.vector.tensor_copy(EW[:, p, 0:512], ewp[:])
                else:
                    nc.scalar.copy(EW[:, p, 512:1024], ewp[:])

            for t in range(L):
                gi_t = gip.tile([128, 1536], dt.bfloat16, tag="gi", name=f"gi{t}")
                for q in range(4):
                    nc.sync.dma_start(gi_t[q * 32:(q + 1) * 32, :],
                                      gi_d.ap()[t * 128 + q * 32:t * 128 + (q + 1) * 32, :])
                if t > 0:
                    hsl = hTr[:, :, ((t - 1) % 4) * BL:((t - 1) % 4 + 1) * BL]
                    ps = encps.tile([128, 1536], dt.float32, tag="g", name=f"g{t}")
                    for g in range(3):
                        for k in range(KH):
                            for hc in range(2):
                                nc.tensor.matmul(
                                    ps[hc * 64:(hc + 1) * 64, g * 512:(g + 1) * 512],
                                    hsl[:, k, :], ksl(encWhhT, k, g * H + hc * 512),
                                    start=(k == 0),
                                    stop=(k == KH - 1 and not (g == 2 and "ebhn" in bias_t)))
                        if g == 2 and "ebhn" in bias_t:
                            for hc in range(2):
                                nc.tensor.matmul(ps[hc * 64:(hc + 1) * 64, 1024:1536],
                                                 ones_sb[:],
                                                 bias_t["ebhn"][0:1, hc * 512:hc * 512 + 512],
                                                 start=False, stop=True)
                if t >= 2:
                    ew_half((t - 2) // 2, (t - 2) % 2)
                if t == 0:
                    # h(-1) = 0: gates come straight from gi (gh ≡ 0, zh ≡ 0)
                    z_s = work.tile([128, 512], dt.float32, tag="z")
                    nc.scalar.activation(z_s[:], gi_t[:, 512:1024], AF.Sigmoid)
                    omz = work.tile([128, 512], dt.float32, tag="omz")
                    nc.gpsimd.tensor_scalar(omz[:], z_s[:], -1.0, 1.0, op0=ALU.mult, op1=ALU.add)
                    n_s = work.tile([128, 512], dt.float32, tag="n")
                    nc.scalar.activation(n_s[:], gi_t[:, 1024:1536], AF.Tanh)
                    nc.vector.tensor_tensor(hst[:], n_s[:], omz[:], op=ALU.mult)
                else:
                    # gates (all ops full 128-partition width)
                    r_s = work.tile([128, 512], dt.float32, tag="r")
                    nc.vector.tensor_tensor(r_s[:], ps[:, 0:512], gi_t[:, 0:512], op=ALU.add)
                    nc.scalar.activation(r_s[:], r_s[:], AF.Sigmoid)
                    z_s = work.tile([128, 512], dt.float32, tag="z")
                    nc.vector.tensor_tensor(z_s[:], ps[:, 512:1024], gi_t[:, 512:1024], op=ALU.add)
                    nc.scalar.activation(z_s[:], z_s[:], AF.Sigmoid)
                    omz = work.tile([128, 512], dt.float32, tag="omz")
                    nc.gpsimd.tensor_scalar(omz[:], z_s[:], -1.0, 1.0, op0=ALU.mult, op1=ALU.add)
                    zh = work.tile([128, 512], dt.float32, tag="zh")
                    nc.vector.tensor_tensor(zh[:], z_s[:], hst[:], op=ALU.mult)
                    n_s = work.tile([128, 512], dt.float32, tag="n")
                    nc.vector.tensor_tensor(n_s[:], ps[:, 1024:1536], r_s[:], op=ALU.mult)
                    nc.vector.tensor_tensor(n_s[:], n_s[:], gi_t[:, 1024:1536], op=ALU.add)
                    nc.scalar.activation(n_s[:], n_s[:], AF.Tanh)
                    for h2 in range(2):
                        c = slice(h2 * 256, (h2 + 1) * 256)
                        nc.vector.tensor_tensor(n_s[:, c], n_s[:, c], omz[:, c], op=ALU.mult)
                        nc.vector.tensor_tensor(hst[:, c], n_s[:, c], zh[:, c], op=ALU.add)
                tpt = tpps.tile([128, KH, BL], dt.bfloat16, tag="tp", name=f"tp{t}")
                stg = work.tile([BL, 512], dt.bfloat16, tag="stg", name=f"stg{t}")
                dst = hTr[:, :, (t % 4) * BL:(t % 4 + 1) * BL]
                transposes(tpt, hst, stg, dst[:, 0:4, :], dst[:, 4:8, :])

            # tail EW halves; EW pair 12 = [enc_out(24); zeros]
            ew_half(11, 1)
            nc.vector.tensor_copy(hTab[0][:], hTr[:, :, 0:BL])
            nc.gpsimd.memset(hTr[:, :, BL:2 * BL], 0.0)
            ew_half(12, 0)
            ew_half(12, 1)
            nc.vector.tensor_copy(combW01[:], combWT[:, 0:2, :])

        # ================= decoder phase =================
        with tc.tile_pool(name="decw", bufs=1) as decw, \
             tc.tile_pool(name="mainps", bufs=3, space="PSUM") as mainps, \
             tc.tile_pool(name="tinyps", bufs=1, space="PSUM") as tinyps, \
             tc.tile_pool(name="tpps2", bufs=1, space="PSUM") as tpps2:
            decWihT = decw.tile([128, KH, G3], dt.bfloat16, tag="decWihT")
            dir_ = decWihT_d.ap().rearrange("(k p) n -> p k n", p=128)
            for k in range(KH):
                nc.sync.dma_start(decWihT[:, k, :], dir_[:, k, :])
            oT = decw.tile([128, KH, BL], dt.bfloat16, tag="oT")
            dgs = decw.tile([128, NP, BL], dt.bfloat16, tag="dgs")
            awsh = decw.tile([128, L], dt.float32, tag="awsh")
            nc.vector.memset(awsh[BL:128, L - 1:L], 0.0)
            out_r = out_d.ap().rearrange("(b l) c -> b l c", l=L)
            # one bank shared by scores / logits / emb psums (disjoint ranges)
            tiny = tinyps.tile([128, 512], dt.float32, tag="tiny")
            sc = tiny[0:BL, 0:L]
            lp = tiny[0:BL, 128:256]
            ep = [tiny[:, 256:320], tiny[:, 320:384]]

            for t in range(L):
                hTc = hTab[t % 2]
                hTn = hTab[(t + 1) % 2]
                # --- phase A ---
                rz = mainps.tile([128, 1024], dt.float32, tag="m", name=f"rz_{t}")
                for g in range(2):      # r block then z block
                    for k in range(KH):
                        for hc in range(2):
                            nc.tensor.matmul(
                                rz[hc * 64:(hc + 1) * 64, g * 512:(g + 1) * 512],
                                hTc[:, k, :], ksl(decWhhT, k, g * H + hc * 512),
                                start=(k == 0), stop=False)
                    if g == 0:
                        # attention scores between the r and z blocks
                        for kt in range(KC):
                            lhs = embT[:, kt, :] if kt < 2 else hTc[:, kt - 2, :]
                            nc.tensor.matmul(sc, lhs, attnWT[:, kt, :],
                                             start=(kt == 0),
                                             stop=(kt == KC - 1 and "attnb" not in bias_t))
                        if "attnb" in bias_t:
                            nc.tensor.matmul(sc, ones_sb[:], bias_t["attnb"][:],
                                             start=False, stop=True)
                # softmax via tanh: exp(x) = (1+tanh(x/2))/(1-tanh(x/2)), x = s-mx
                mx = small.tile([BL, 1], dt.float32, tag="mx")
                nc.vector.tensor_reduce(mx[:], sc, axis=AX.X, op=ALU.max)
                nmxh = small.tile([BL, 1], dt.float32, tag="nmxh")
                nc.vector.tensor_scalar(nmxh[:], mx[:], -0.5, None, op0=ALU.mult)
                tt = small.tile([BL, L], dt.float32, tag="tt")
                nc.scalar.activation(tt[:], sc, AF.Tanh, scale=0.5, bias=nmxh[:])
                num = small.tile([BL, L], dt.float32, tag="num")
                nc.gpsimd.tensor_scalar(num[:], tt[:], 1.0, None, op0=ALU.add)
                den = small.tile([BL, L], dt.float32, tag="den")
                nc.vector.tensor_scalar(den[:], tt[:], -1.0, 1.0, op0=ALU.mult, op1=ALU.add)
                rcp = small.tile([BL, L], dt.float32, tag="rcp")
                nc.vector.reciprocal(rcp[:], den[:])
                e = small.tile([BL, L], dt.float32, tag="e")
                nc.vector.tensor_tensor(e[:], num[:], rcp[:], op=ALU.mult)
                sume = small.tile([BL, 1], dt.float32, tag="sume")
                nc.vector.tensor_reduce(sume[:], e[:], axis=AX.X, op=ALU.add)
                rs = small.tile([BL, 1], dt.float32, tag="rs")
                nc.vector.reciprocal(rs[:], sume[:])
                en = small.tile([BL, L], dt.float32, tag="en")
                nc.vector.tensor_scalar(en[:], e[:], rs[:], None, op0=ALU.mult)
                nc.vector.tensor_copy(awsh[0:BL, :], en[:])
                nc.gpsimd.tensor_copy(awsh[BL:128, 0:L - 1], en[:, 1:L])
                for p in range(NP):
                    nc.vector.tensor_scalar(dgs[:, p, :], Istk[:], awsh[:, 2 * p:2 * p + 1],
                                            None, op0=ALU.mult)
                # comb psum: emb part + EW-diag + optional bias
                cb = mainps.tile([BL, H], dt.float32, tag="m", name=f"cb_{t}")
                for kt in range(2):
                    for n2 in range(2):
                        nc.tensor.matmul(cb[:, n2 * 512:(n2 + 1) * 512], embT[:, kt, :],
                                         ksl(combW01, kt, n2 * 512),
                                         start=(kt == 0), stop=False)
                for p in range(NP):
                    for n2 in range(2):
                        nc.tensor.matmul(
                            cb[:, n2 * 512:(n2 + 1) * 512], dgs[:, p, :],
                            EW[:, p, n2 * 512:(n2 + 1) * 512], start=False,
                            stop=(p == NP - 1 and "combb" not in bias_t))
                if "combb" in bias_t:
                    for n2 in range(2):
                        nc.tensor.matmul(cb[:, n2 * 512:(n2 + 1) * 512], ones_sb[:],
                                         bias_t["combb"][0:1, n2 * 512:n2 * 512 + 512],
                                         start=False, stop=True)
                # n-gate Whh part fills the PE while relu/oT resolve
                nn = mainps.tile([128, 1024], dt.float32, tag="m", name=f"nn_{t}")
                for k in range(KH):
                    for hc in range(2):
                        nc.tensor.matmul(nn[hc * 64:(hc + 1) * 64, 0:512], hTc[:, k, :],
                                         ksl(decWhhT, k, 2 * H + hc * 512),
                                         start=(k == 0),
                                         stop=(k == KH - 1 and "dbhn" not in bias_t))
                # o = relu(s2 * cb); oT via PE transpose
                obf = work.tile([BL, H], dt.bfloat16, tag="obf")
                nc.scalar.activation(obf[:], cb[:], AF.Relu, scale=s2_scale)
                tpo = tpps2.tile([128, KH + 1, BL], dt.bfloat16, tag="tp2",
                                 name=f"tpo_{t}")
                for k in range(4):
                    nc.tensor.transpose(tpo[:, k, :], obf[:, k * 128:(k + 1) * 128],
                                        Istk[0:BL, :])
                nc.vector.tensor_copy(oT[:, 0:4, :], tpo[:, 0:4, :])
                for k in range(4, KH):
                    nc.tensor.transpose(tpo[:, k, :], obf[:, k * 128:(k + 1) * 128],
                                        Istk[0:BL, :])
                nc.scalar.copy(oT[:, 4:8, :], tpo[:, 4:8, :])
                # --- phase B: Wih for r,z; sigmoids; n gates; h2 ---
                for g in range(2):
                    for k in range(KH):
                        for hc in range(2):
                            nc.tensor.matmul(
                                rz[hc * 64:(hc + 1) * 64, g * 512:(g + 1) * 512],
                                oT[:, k, :], ksl(decWihT, k, g * H + hc * 512),
                                start=False,
                                stop=(k == KH - 1 and "dgibrz" not in bias_t))
                    if "dgibrz" in bias_t:
                        for hc in range(2):
                            nc.tensor.matmul(
                                rz[hc * 64:(hc + 1) * 64, g * 512:(g + 1) * 512],
                                ones_sb[:],
                                bias_t["dgibrz"][0:1, g * H + hc * 512:g * H + hc * 512 + 512],
                                start=False, stop=True)
                r_s = work.tile([128, 512], dt.float32, tag="r")
                nc.scalar.activation(r_s[:], rz[:, 0:512], AF.Sigmoid)
                z_s = work.tile([128, 512], dt.float32, tag="z")
                nc.scalar.activation(z_s[:], rz[:, 512:1024], AF.Sigmoid)
                omz = work.tile([128, 512], dt.float32, tag="omz")
                nc.gpsimd.tensor_scalar(omz[:], z_s[:], -1.0, 1.0, op0=ALU.mult, op1=ALU.add)
                zh = work.tile([128, 512], dt.float32, tag="zh")
                nc.vector.tensor_tensor(zh[:], z_s[:], hst[:], op=ALU.mult)
                for k in range(KH):
                    for hc in range(2):
                        nc.tensor.matmul(nn[hc * 64:(hc + 1) * 64, 512:1024], oT[:, k, :],
                                         ksl(decWihT, k, 2 * H + hc * 512),
                                         start=(k == 0),
                                         stop=(k == KH - 1 and "dgibn" not in bias_t))
                for hc in range(2):
                    if "dbhn" in bias_t:
                        nc.tensor.matmul(nn[hc * 64:(hc + 1) * 64, 0:512], ones_sb[:],
                                         bias_t["dbhn"][0:1, hc * 512:hc * 512 + 512],
                                         start=False, stop=True)
                    if "dgibn" in bias_t:
                        nc.tensor.matmul(nn[hc * 64:(hc + 1) * 64, 512:1024], ones_sb[:],
                                         bias_t["dgibn"][0:1, hc * 512:hc * 512 + 512],
                                         start=False, stop=True)
                n_s = work.tile([128, 512], dt.float32, tag="n")
                nc.vector.tensor_tensor(n_s[:], nn[:, 0:512], r_s[:], op=ALU.mult)
                nc.vector.tensor_tensor(n_s[:], n_s[:], nn[:, 512:1024], op=ALU.add)
                nc.scalar.activation(n_s[:], n_s[:], AF.Tanh)
                for h2 in range(2):
                    c = slice(h2 * 256, (h2 + 1) * 256)
                    nc.vector.tensor_tensor(n_s[:, c], n_s[:, c], omz[:, c], op=ALU.mult)
                    nc.vector.tensor_tensor(hst[:, c], n_s[:, c], zh[:, c], op=ALU.add)
                tph = tpps2.tile([128, KH + 1, BL], dt.bfloat16, tag="tp2",
                                 name=f"tph_{t}")
                stg = work.tile([BL, 512], dt.bfloat16, tag="stg", name=f"dstg{t}")
                transposes(tph[:, 0:KH, :], hst, stg, hTn[:, 0:4, :], hTn[:, 4:8, :])
                # --- logits, argmax, next embedding, exp-sum for log-softmax ---
                for k in range(KH):
                    nc.tensor.matmul(lp, hTn[:, k, :], outWTs[:, k, :],
                                     start=(k == 0),
                                     stop=(k == KH - 1 and "lgb" not in bias_t))
                if "lgb" in bias_t:
                    nc.tensor.matmul(lp, ones_sb[:], bias_t["lgb"][:],
                                     start=False, stop=True)
                lgt = small.tile([BL, T], dt.float32, tag="lg", name=f"lg{t}")
                lg = lgt[:]
                nc.vector.tensor_copy(lg, lp)
                nc.sync.dma_start(out_r[:, t, :], lg)
                if t < L - 1:
                    mx2 = small.tile([BL, 1], dt.float32, tag="mx2")
                    nc.vector.tensor_reduce(mx2[:], lg, axis=AX.X, op=ALU.max)
                    oh = small.tile([BL, T], dt.bfloat16, tag="oh")
                    nc.vector.tensor_scalar(oh[:], lg, mx2[:], None, op0=ALU.is_equal)
                    ohp = tpps2.tile([128, KH + 1, BL], dt.bfloat16, tag="tp2",
                                     name=f"ohp_{t}")
                    nc.tensor.transpose(ohp[:, 0, :], oh[:], Istk[0:BL, :])
                    ohT = small.tile([128, BL], dt.bfloat16, tag="ohT")
                    nc.vector.tensor_copy(ohT[:], ohp[:, 0, :])
                    for d2 in range(2):
                        nc.tensor.matmul(ep[d2], embbf[:, d2 * 128:(d2 + 1) * 128],
                                         ohT[:], start=True, stop=True)
                    nc.vector.tensor_copy(embT[:, 0, :], ep[0])
                    nc.scalar.copy(embT[:, 1, :], ep[1])
    nc.finalize()
    return nc


def kernel(**inputs):
    import concourse.bass_utils as bass_utils

    tokens = np.asarray(inputs["tokens"])
    w2v = np.asarray(inputs["w2v"], np.float32)
    bn1 = np.asarray(inputs["bn1"], np.float32)
    bn2 = np.asarray(inputs["bn2"], np.float32)
    s1 = float(bn1[0] / np.sqrt(bn1[3] + BN_EPS))
    t1 = float(bn1[1] - bn1[2] * s1)
    s2 = float(bn2[0] / np.sqrt(bn2[3] + BN_EPS))
    t2 = float(bn2[1] - bn2[2] * s2)

    f32 = lambda k: np.asarray(inputs[k], np.float32)
    bft = lambda a: np.ascontiguousarray(np.asarray(a, np.float32).T).astype(BF16)
    enc_bih, enc_bhh = f32("enc_bih"), f32("enc_bhh")
    dec_bih, dec_bhh = f32("dec_bih"), f32("dec_bhh")
    out_W = f32("out_W")
    lgb = (f32("out_b") + t1 * out_W.sum(axis=1))[None, :]
    combb = (f32("comb_b") + t2 / s2)[None, :]

    def opt_bias(row):  # ship only if nonzero
        return None if np.all(row == 0.0) else np.ascontiguousarray(row).astype(BF16)

    biases = {
        "ebhn": opt_bias(enc_bhh[2 * H:][None, :]),
        "dgibrz": opt_bias((dec_bih[:2 * H] + dec_bhh[:2 * H])[None, :]),
        "dbhn": opt_bias(dec_bhh[2 * H:][None, :]),
        "dgibn": opt_bias(dec_bih[2 * H:][None, :]),
        "combb": opt_bias(combb),
        "attnb": opt_bias(f32("attn_b")[None, :]),
        "lgb": opt_bias(lgb),
    }

    # dec_emb renorm (max_norm=1): rows 0..127 for the lookup, row 128 = SOS
    dec_emb = f32("dec_emb")
    nrm = np.linalg.norm(dec_emb, axis=-1, keepdims=True)
    emb_rn = dec_emb * np.minimum(1.0, MAXN2 / (nrm + 1e-7))
    embbf = np.ascontiguousarray(emb_rn[:T]).astype(BF16)
    sos = emb_rn[T]  # (256,)
    sosT = np.zeros((128, 2 * BL), np.float32)
    for k in range(2):
        sosT[:, k * BL:(k + 1) * BL] = sos[k * 128:(k + 1) * 128][:, None]

    istk = np.zeros((128, BL), np.float32)
    istk[np.arange(128), np.arange(128) % BL] = 1.0

    common = {
        "encWhhT": bft(inputs["enc_Whh"]), "decWihT": bft(inputs["dec_Wih"]),
        "decWhhT": bft(inputs["dec_Whh"]), "combWT": bft(inputs["comb_W"]),
        "outWTs": np.ascontiguousarray((s1 * out_W).T).astype(BF16),
        "attnWT": bft(inputs["attn_W"]),
        "embbf": embbf, "sosT": sosT.astype(BF16), "istk": istk.astype(BF16),
    }
    for k, v in biases.items():
        if v is not None:
            common[k] = v

    # host-side encoder input projection, rows = t*128 + hc*64 + b
    enc_Wih = f32("enc_Wih")
    gi_bias = np.concatenate([
        enc_bih[:H] + enc_bhh[:H], enc_bih[H:2 * H] + enc_bhh[H:2 * H],
        enc_bih[2 * H:]])  # (3072,)
    in_maps = []
    for c in range(NC):
        tok = tokens[c * BL:(c + 1) * BL].astype(np.int64)   # (64,25)
        x = w2v[tok]                                         # (64,25,300)
        n = np.linalg.norm(x, axis=-1, keepdims=True)
        x = x * np.minimum(1.0, MAXN1 / (n + 1e-7))
        gi = x.astype(np.float32) @ enc_Wih.T + gi_bias      # (64,25,3072)
        gi = gi.transpose(1, 0, 2).reshape(L, BL, 3, 2, 512)
        gi = gi.transpose(0, 3, 1, 2, 4).reshape(L * 2 * BL, 1536)
        m = dict(common)
        m["gi"] = np.ascontiguousarray(gi).astype(BF16)
        in_maps.append(m)

    nc = build_nc(s2, biases)
    trace = bool(int(os.environ.get("KERNEL_TRACE", "0")))
    res = bass_utils.run_bass_kernel_spmd(nc, in_maps, core_ids=list(range(NC)),
                                          trace=trace)
    if trace and res.exec_time_ns is not None:
        print(f"HW exec time: {res.exec_time_ns} ns", flush=True)
        print("trace:", res.instructions_and_trace[1] if res.instructions_and_trace else None,
              flush=True)
    lg = np.concatenate([res.results[c]["out"] for c in range(NC)], axis=0)
    lg = lg.astype(np.float32)
    mx = lg.max(axis=-1, keepdims=True)
    lse = mx + np.log(np.exp(lg - mx).sum(axis=-1, keepdims=True))
    return lg - lse


if __name__ == "__main__":
    pass


# revision 20
# speedup vs baseline: 1.0263x; 1.0263x over previous
"""Trainium2 Bass kernel for nn_Attention_72670846649042.

GRU encoder + greedy attention decoder, B=512,L=25,H=1024,D=256,T=128,E=300.
Sharding: data-parallel over batch, 64 rows/core on 8 cores, no collectives.

v3 design:
 - Host precomputes the encoder input projection gi (bf16) and all layout
   transforms; device runs only the two recurrences.
 - Partition-packed elementwise: gate PSUMs are (128, 512) holding both
   512-column halves of the hidden dim on partition ranges [0:64)/[64:128)
   (matmul quadrant tile_position), so every gate op runs at full DVE width.
 - GRU state is a single persistent bf16 (128,512) tile; h2 = zh + (1-z)*n
   with zh/(1-z) precomputed off the critical chain.
 - hT tiles rebuilt via PE transposes (identity matmul); rolling 4-slot hT
   history feeds the pairwise EW precompute (EW = enc_out @ comb_W2.T)
   interleaved into the encoder; per decoder step attention-apply + comb
   collapse into one PSUM-accumulated block-diag matmul over EW.
 - All softmaxes use exp(x)=(1+tanh(x/2))/(1-tanh(x/2)) with max subtraction;
   log-softmax denominators deferred to one Ln at the end. The whole loop
   uses one activation table (sigmoid/tanh/relu/copy).
 - Biases in this instance are all zero; bias matmuls emitted only if nonzero.
"""
import os
import numpy as np
import ml_dtypes

B, L, V, E, H, D, T = 512, 25, 50000, 300, 1024, 256, 128
NC = 8
BL = B // NC          # 64 local batch
G3 = 3 * H            # 3072
KH = H // 128         # 8 hidden ktiles
KC = (D + H) // 128   # 10 ktiles for concat(emb, h/applied)
NP = 13               # l-pairs (2 l per 128-row K tile); l=25 is zero-padded
MAXN1, MAXN2, BN_EPS = 10.0, 1.0, 1e-5
BF16 = ml_dtypes.bfloat16

LINEARIZE = False


def build_nc(s2_scale, biases):
    """biases: dict name -> np row [1,X] or None (zero => op not emitted)."""
    import concourse.bass as bass
    import concourse.tile as tile
    from concourse import bacc, mybir
    from contextlib import ExitStack

    dt = mybir.dt
    AF = mybir.ActivationFunctionType
    ALU = mybir.AluOpType
    AX = mybir.AxisListType

    nc = bacc.Bacc("TRN2", target_bir_lowering=False, debug=False)

    # ---- dram parameters ----
    # gi rows: t*128 + hc*64 + b, cols [r|z|n] (512 each) for that hc
    gi_d = nc.declare_dram_parameter("gi", [L * 2 * BL, 1536], dt.bfloat16, isOutput=False)
    encWhhT_d = nc.declare_dram_parameter("encWhhT", [H, G3], dt.bfloat16, isOutput=False)
    decWihT_d = nc.declare_dram_parameter("decWihT", [H, G3], dt.bfloat16, isOutput=False)
    decWhhT_d = nc.declare_dram_parameter("decWhhT", [H, G3], dt.bfloat16, isOutput=False)
    combWT_d = nc.declare_dram_parameter("combWT", [D + H, H], dt.bfloat16, isOutput=False)
    outWTs_d = nc.declare_dram_parameter("outWTs", [H, T], dt.bfloat16, isOutput=False)
    attnWT_d = nc.declare_dram_parameter("attnWT", [D + H, L], dt.bfloat16, isOutput=False)
    embbf_d = nc.declare_dram_parameter("embbf", [128, D], dt.bfloat16, isOutput=False)
    sosT_d = nc.declare_dram_parameter("sosT", [128, 2 * BL], dt.bfloat16, isOutput=False)
    istk_d = nc.declare_dram_parameter("istk", [128, BL], dt.bfloat16, isOutput=False)
    bias_d = {}
    for k, v in biases.items():
        if v is not None:
            bias_d[k] = nc.declare_dram_parameter(k, list(v.shape), dt.bfloat16,
                                                  isOutput=False)
    out_d = nc.declare_dram_parameter("out", [BL * L, T], dt.float32, isOutput=True)

    with tile.TileContext(nc, linearize=LINEARIZE) as tc, ExitStack() as ctx:
        # ================= static pools =================
        shared = ctx.enter_context(tc.tile_pool(name="shared", bufs=1))
        work = ctx.enter_context(tc.tile_pool(name="work", bufs=2))
        small = ctx.enter_context(tc.tile_pool(name="small", bufs=2))

        decWhhT = shared.tile([128, KH, G3], dt.bfloat16, tag="decWhhT")
        EW = shared.tile([128, NP, H], dt.bfloat16, tag="EW")
        hst = shared.tile([128, 512], dt.bfloat16, tag="hst")  # h state, packed
        Istk = shared.tile([128, BL], dt.bfloat16, tag="Istk")
        combW01 = shared.tile([128, 2, H], dt.bfloat16, tag="combW01")
        hTab = [shared.tile([128, KH, BL], dt.bfloat16, tag=f"hT{i}", name=f"hT{i}")
                for i in range(2)]
        outWTs = shared.tile([128, KH, T], dt.bfloat16, tag="outWTs")
        attnWT = shared.tile([128, KC, L], dt.bfloat16, tag="attnWT")
        embbf = shared.tile([128, D], dt.bfloat16, tag="embbf")
        embT = shared.tile([128, 2, BL], dt.bfloat16, tag="embT")
        bias_t = {}
        for k in bias_d:
            bias_t[k] = shared.tile(list(biases[k].shape), dt.bfloat16, tag=k, name=k)
            nc.sync.dma_start(bias_t[k][:], bias_d[k].ap())
        if bias_d:
            ones_sb = shared.tile([1, BL], dt.bfloat16, tag="ones_sb")
            nc.vector.memset(ones_sb[:], 1.0)

        # small DMAs first so they don't queue behind the big weights
        nc.sync.dma_start(Istk[:], istk_d.ap())
        nc.sync.dma_start(embT[:], sosT_d.ap().rearrange("p (k b) -> p k b", b=BL))
        nc.sync.dma_start(embbf[:], embbf_d.ap())
        nc.sync.dma_start(attnWT[:], attnWT_d.ap().rearrange("(k p) n -> p k n", p=128))
        nc.sync.dma_start(outWTs[:], outWTs_d.ap().rearrange("(k p) n -> p k n", p=128))
        nc.vector.memset(hst[:], 0.0)

        def ksl(w, k, lo, n=512):  # weight tile slice helper
            return w[:, k, lo:lo + n]

        def transposes(tpt, src, stg, dstv, dsts):
            """PE-transpose packed (128,512) bf16 src into [128, k, 64] dst.
            The PE cannot switch lhsT partition base between ops, so the upper
            half is staged to partitions 0-63 via SBUF-SBUF DMA first."""
            for q in range(4):
                eng = nc.sync if q % 2 == 0 else nc.scalar
                eng.dma_start(stg[:, q * 128:(q + 1) * 128],
                              src[64:128, q * 128:(q + 1) * 128])
            for k in range(4):
                nc.tensor.transpose(tpt[:, k, :], src[0:64, k * 128:(k + 1) * 128],
                                    Istk[0:BL, :])
            nc.vector.tensor_copy(dstv, tpt[:, 0:4, :])
            for k in range(4, KH):
                nc.tensor.transpose(tpt[:, k, :], stg[:, (k - 4) * 128:(k - 3) * 128],
                                    Istk[0:BL, :])
            nc.scalar.copy(dsts, tpt[:, 4:8, :])

        # ================= encoder phase =================
        with tc.tile_pool(name="encw", bufs=1) as encw, \
             tc.tile_pool(name="gip", bufs=3) as gip, \
             tc.tile_pool(name="encps", bufs=2, space="PSUM") as encps, \
             tc.tile_pool(name="tpps", bufs=1, space="PSUM") as tpps, \
             tc.tile_pool(name="ewps", bufs=1, space="PSUM") as ewps:
            encWhhT = encw.tile([128, KH, G3], dt.bfloat16, tag="encWhhT")
            for k in range(KH):
                for h4 in range(4):
                    nc.sync.dma_start(encWhhT[:, k, h4 * 768:(h4 + 1) * 768],
                                      encWhhT_d.ap()[k * 128:(k + 1) * 128,
                                                     h4 * 768:(h4 + 1) * 768])
            combWT = encw.tile([128, KC, H], dt.bfloat16, tag="combWT")
            cwr = combWT_d.ap().rearrange("(k p) n -> p k n", p=128)
            for k in range(KC):
                nc.sync.dma_start(combWT[:, k, :], cwr[:, k, :])
            dwr = decWhhT_d.ap().rearrange("(k p) n -> p k n", p=128)
            for k in range(KH):
                nc.sync.dma_start(decWhhT[:, k, :], dwr[:, k, :])

            # rolling hT history: slot t%4 holds h2T(t); slot 3 = h(-1) = 0
            hTr = encw.tile([128, KH, 4 * BL], dt.bfloat16, tag="hTr")
            nc.vector.memset(hTr[:, :, 3 * BL:4 * BL], 0.0)

            def ew_half(p, n2):
                lo = (2 * p) % 4 * BL
                ewp = ewps.tile([128, 512], dt.float32, tag="ewp")
                for k in range(KH):
                    nc.tensor.matmul(ewp[:], hTr[:, k, lo:lo + 128],
                                     ksl(combWT, 2 + k, n2 * 512),
                                     start=(k == 0), stop=(k == KH - 1))
                if n2 == 0:
                    nc.vector.tensor_copy(EW[:, p, 0:512], ewp[:])
                else:
                    nc.scalar.copy(EW[:, p, 512:1024], ewp[:])

            for t in range(L):
                gi_t = gip.tile([128, 1536], dt.bfloat16, tag="gi", name=f"gi{t}")
                for q in range(4):
                    nc.sync.dma_start(gi_t[q * 32:(q + 1) * 32, :],
                                      gi_d.ap()[t * 128 + q * 32:t * 128 + (q + 1) * 32, :])
                if t > 0:
                    hsl = hTr[:, :, ((t - 1) % 4) * BL:((t - 1) % 4 + 1) * BL]
                    ps = encps.tile([128, 1536], dt.float32, tag="g", name=f"g{t}")
                    for g in range(3):
                        for k in range(KH):
                            for hc in range(2):
                                nc.tensor.matmul(
                                    ps[hc * 64:(hc + 1) * 64, g * 512:(g + 1) * 512],
                                    hsl[:, k, :], ksl(encWhhT, k, g * H + hc * 512),
                                    start=(k == 0),
                                    stop=(k == KH - 1 and not (g == 2 and "ebhn" in bias_t)))
                        if g == 2 and "ebhn" in bias_t:
                            for hc in range(2):
                                nc.tensor.matmul(ps[hc * 64:(hc + 1) * 64, 1024:1536],
                                                 ones_sb[:],
                                                 bias_t["ebhn"][0:1, hc * 512:hc * 512 + 512],
                                                 start=False, stop=True)
                if t >= 2:
                    ew_half((t - 2) // 2, (t - 2) % 2)
                if t == 0:
                    # h(-1) = 0: gates come straight from gi (gh ≡ 0, zh ≡ 0)
                    z_s = work.tile([128, 512], dt.float32, tag="z")
                    nc.scalar.activation(z_s[:], gi_t[:, 512:1024], AF.Sigmoid)
                    omz = work.tile([128, 512], dt.float32, tag="omz")
                    nc.gpsimd.tensor_scalar(omz[:], z_s[:], -1.0, 1.0, op0=ALU.mult, op1=ALU.add)
                    n_s = work.tile([128, 512], dt.float32, tag="n")
                    nc.scalar.activation(n_s[:], gi_t[:, 1024:1536], AF.Tanh)
                    nc.vector.tensor_tensor(hst[:], n_s[:], omz[:], op=ALU.mult)
                else:
                    # gates (all ops full 128-partition width)
                    r_s = work.tile([128, 512], dt.float32, tag="r")
                    nc.vector.tensor_tensor(r_s[:], ps[:, 0:512], gi_t[:, 0:512], op=ALU.add)
                    nc.scalar.activation(r_s[:], r_s[:], AF.Sigmoid)
                    z_s = work.tile([128, 512], dt.float32, tag="z")
                    nc.vector.tensor_tensor(z_s[:], ps[:, 512:1024], gi_t[:, 512:1024], op=ALU.add)
                    nc.scalar.activation(z_s[:], z_s[:], AF.Sigmoid)
                    omz = work.tile([128, 512], dt.float32, tag="omz")
                    nc.gpsimd.tensor_scalar(omz[:], z_s[:], -1.0, 1.0, op0=ALU.mult, op1=ALU.add)
                    zh = work.tile([128, 512], dt.float32, tag="zh")
                    nc.vector.tensor_tensor(zh[:], z_s[:], hst[:], op=ALU.mult)
                    n_s = work.tile([128, 512], dt.float32, tag="n")
                    nc.vector.tensor_tensor(n_s[:], ps[:, 1024:1536], r_s[:], op=ALU.mult)
                    nc.vector.tensor_tensor(n_s[:], n_s[:], gi_t[:, 1024:1536], op=ALU.add)
                    nc.scalar.activation(n_s[:], n_s[:], AF.Tanh)
                    nc.vector.tensor_tensor(n_s[:], n_s[:], omz[:], op=ALU.mult)
                    nc.vector.tensor_tensor(hst[:], n_s[:], zh[:], op=ALU.add)
                tpt = tpps.tile([128, KH, BL], dt.bfloat16, tag="tp", name=f"tp{t}")
                stg = work.tile([BL, 512], dt.bfloat16, tag="stg", name=f"stg{t}")
                dst = hTr[:, :, (t % 4) * BL:(t % 4 + 1) * BL]
                transposes(tpt, hst, stg, dst[:, 0:4, :], dst[:, 4:8, :])

            # tail EW halves; EW pair 12 = [enc_out(24); zeros]
            ew_half(11, 1)
            nc.vector.tensor_copy(hTab[0][:], hTr[:, :, 0:BL])
            nc.gpsimd.memset(hTr[:, :, BL:2 * BL], 0.0)
            ew_half(12, 0)
            ew_half(12, 1)
            nc.vector.tensor_copy(combW01[:], combWT[:, 0:2, :])

        # ================= decoder phase =================
        with tc.tile_pool(name="decw", bufs=1) as decw, \
             tc.tile_pool(name="mainps", bufs=3, space="PSUM") as mainps, \
             tc.tile_pool(name="tinyps", bufs=1, space="PSUM") as tinyps, \
             tc.tile_pool(name="tpps2", bufs=1, space="PSUM") as tpps2:
            decWihT = decw.tile([128, KH, G3], dt.bfloat16, tag="decWihT")
            dir_ = decWihT_d.ap().rearrange("(k p) n -> p k n", p=128)
            for k in range(KH):
                nc.sync.dma_start(decWihT[:, k, :], dir_[:, k, :])
            oT = decw.tile([128, KH, BL], dt.bfloat16, tag="oT")
            dgs = decw.tile([128, NP, BL], dt.bfloat16, tag="dgs")
            awsh = decw.tile([128, L], dt.float32, tag="awsh")
            nc.vector.memset(awsh[BL:128, L - 1:L], 0.0)
            out_r = out_d.ap().rearrange("(b l) c -> b l c", l=L)
            # one bank shared by scores / logits / emb psums (disjoint ranges)
            tiny = tinyps.tile([128, 512], dt.float32, tag="tiny")
            sc = tiny[0:BL, 0:L]
            lp = tiny[0:BL, 128:256]
            ep = [tiny[:, 256:320], tiny[:, 320:384]]

            for t in range(L):
                hTc = hTab[t % 2]
                hTn = hTab[(t + 1) % 2]
                # --- phase A ---
                rz = mainps.tile([128, 1024], dt.float32, tag="m", name=f"rz_{t}")
                for g in range(2):      # r block then z block
                    for k in range(KH):
                        for hc in range(2):
                            nc.tensor.matmul(
                                rz[hc * 64:(hc + 1) * 64, g * 512:(g + 1) * 512],
                                hTc[:, k, :], ksl(decWhhT, k, g * H + hc * 512),
                                start=(k == 0), stop=False)
                    if g == 0:
                        # attention scores between the r and z blocks
                        for kt in range(KC):
                            lhs = embT[:, kt, :] if kt < 2 else hTc[:, kt - 2, :]
                            nc.tensor.matmul(sc, lhs, attnWT[:, kt, :],
                                             start=(kt == 0),
                                             stop=(kt == KC - 1 and "attnb" not in bias_t))
                        if "attnb" in bias_t:
                            nc.tensor.matmul(sc, ones_sb[:], bias_t["attnb"][:],
                                             start=False, stop=True)
                # softmax via tanh: exp(x) = (1+tanh(x/2))/(1-tanh(x/2)), x = s-mx
                mx = small.tile([BL, 1], dt.float32, tag="mx")
                nc.vector.tensor_reduce(mx[:], sc, axis=AX.X, op=ALU.max)
                nmxh = small.tile([BL, 1], dt.float32, tag="nmxh")
                nc.vector.tensor_scalar(nmxh[:], mx[:], -0.5, None, op0=ALU.mult)
                tt = small.tile([BL, L], dt.float32, tag="tt")
                nc.scalar.activation(tt[:], sc, AF.Tanh, scale=0.5, bias=nmxh[:])
                num = small.tile([BL, L], dt.float32, tag="num")
                nc.gpsimd.tensor_scalar(num[:], tt[:], 1.0, None, op0=ALU.add)
                den = small.tile([BL, L], dt.float32, tag="den")
                nc.vector.tensor_scalar(den[:], tt[:], -1.0, 1.0, op0=ALU.mult, op1=ALU.add)
                rcp = small.tile([BL, L], dt.float32, tag="rcp")
                nc.vector.reciprocal(rcp[:], den[:])
                e = small.tile([BL, L], dt.float32, tag="e")
                nc.vector.tensor_tensor(e[:], num[:], rcp[:], op=ALU.mult)
                sume = small.tile([BL, 1], dt.float32, tag="sume")
                nc.vector.tensor_reduce(sume[:], e[:], axis=AX.X, op=ALU.add)
                rs = small.tile([BL, 1], dt.float32, tag="rs")
                nc.vector.reciprocal(rs[:], sume[:])
                en = small.tile([BL, L], dt.float32, tag="en")
                nc.vector.tensor_scalar(en[:], e[:], rs[:], None, op0=ALU.mult)
                nc.vector.tensor_copy(awsh[0:BL, :], en[:])
                nc.gpsimd.tensor_copy(awsh[BL:128, 0:L - 1], en[:, 1:L])
                for p in range(NP):
                    nc.vector.tensor_scalar(dgs[:, p, :], Istk[:], awsh[:, 2 * p:2 * p + 1],
                                            None, op0=ALU.mult)
                # comb psum: emb part + EW-diag + optional bias
                cb = mainps.tile([BL, H], dt.float32, tag="m", name=f"cb_{t}")
                for kt in range(2):
                    for n2 in range(2):
                        nc.tensor.matmul(cb[:, n2 * 512:(n2 + 1) * 512], embT[:, kt, :],
                                         ksl(combW01, kt, n2 * 512),
                                         start=(kt == 0), stop=False)
                for p in range(NP):
                    for n2 in range(2):
                        nc.tensor.matmul(
                            cb[:, n2 * 512:(n2 + 1) * 512], dgs[:, p, :],
                            EW[:, p, n2 * 512:(n2 + 1) * 512], start=False,
                            stop=(p == NP - 1 and "combb" not in bias_t))
                if "combb" in bias_t:
                    for n2 in range(2):
                        nc.tensor.matmul(cb[:, n2 * 512:(n2 + 1) * 512], ones_sb[:],
                                         bias_t["combb"][0:1, n2 * 512:n2 * 512 + 512],
                                         start=False, stop=True)
                # n-gate Whh part fills the PE while relu/oT resolve
                nn = mainps.tile([128, 1024], dt.float32, tag="m", name=f"nn_{t}")
                for k in range(KH):
                    for hc in range(2):
                        nc.tensor.matmul(nn[hc * 64:(hc + 1) * 64, 0:512], hTc[:, k, :],
                                         ksl(decWhhT, k, 2 * H + hc * 512),
                                         start=(k == 0),
                                         stop=(k == KH - 1 and "dbhn" not in bias_t))
                # o = relu(s2 * cb); oT via PE transpose
                obf = work.tile([BL, H], dt.bfloat16, tag="obf")
                nc.scalar.activation(obf[:, 0:512], cb[:, 0:512], AF.Relu, scale=s2_scale)
                nc.scalar.activation(obf[:, 512:1024], cb[:, 512:1024], AF.Relu,
                                     scale=s2_scale)
                tpo = tpps2.tile([128, KH + 1, BL], dt.bfloat16, tag="tp2",
                                 name=f"tpo_{t}")
                for k in range(4):
                    nc.tensor.transpose(tpo[:, k, :], obf[:, k * 128:(k + 1) * 128],
                                        Istk[0:BL, :])
                nc.vector.tensor_copy(oT[:, 0:4, :], tpo[:, 0:4, :])
                for k in range(4, KH):
                    nc.tensor.transpose(tpo[:, k, :], obf[:, k * 128:(k + 1) * 128],
                                        Istk[0:BL, :])
                nc.scalar.copy(oT[:, 4:8, :], tpo[:, 4:8, :])
                # --- phase B: Wih for r,z; sigmoids; n gates; h2 ---
                for g in range(2):
                    for k in range(KH):
                        for hc in range(2):
                            nc.tensor.matmul(
                                rz[hc * 64:(hc + 1) * 64, g * 512:(g + 1) * 512],
                                oT[:, k, :], ksl(decWihT, k, g * H + hc * 512),
                                start=False,
                                stop=(k == KH - 1 and "dgibrz" not in bias_t))
                    if "dgibrz" in bias_t:
                        for hc in range(2):
                            nc.tensor.matmul(
                                rz[hc * 64:(hc + 1) * 64, g * 512:(g + 1) * 512],
                                ones_sb[:],
                                bias_t["dgibrz"][0:1, g * H + hc * 512:g * H + hc * 512 + 512],
                                start=False, stop=True)
                r_s = work.tile([128, 512], dt.float32, tag="r")
                nc.scalar.activation(r_s[:], rz[:, 0:512], AF.Sigmoid)
                z_s = work.tile([128, 512], dt.float32, tag="z")
                nc.scalar.activation(z_s[:], rz[:, 512:1024], AF.Sigmoid)
                omz = work.tile([128, 512], dt.float32, tag="omz")
                nc.vector.tensor_scalar(omz[:], z_s[:], -1.0, 1.0, op0=ALU.mult, op1=ALU.add)
                zh = work.tile([128, 512], dt.float32, tag="zh")
                nc.vector.tensor_tensor(zh[:], z_s[:], hst[:], op=ALU.mult)
                for k in range(KH):
                    for hc in range(2):
                        nc.tensor.matmul(nn[hc * 64:(hc + 1) * 64, 512:1024], oT[:, k, :],
                                         ksl(decWihT, k, 2 * H + hc * 512),
                                         start=(k == 0),
                                         stop=(k == KH - 1 and "dgibn" not in bias_t))
                for hc in range(2):
                    if "dbhn" in bias_t:
                        nc.tensor.matmul(nn[hc * 64:(hc + 1) * 64, 0:512], ones_sb[:],
                                         bias_t["dbhn"][0:1, hc * 512:hc * 512 + 512],
                                         start=False, stop=True)
                    if "dgibn" in bias_t:
                        nc.tensor.matmul(nn[hc * 64:(hc + 1) * 64, 512:1024], ones_sb[:],
                                         bias_t["dgibn"][0:1, hc * 512:hc * 512 + 512],
                                         start=False, stop=True)
                n_s = work.tile([128, 512], dt.float32, tag="n")
                nc.vector.tensor_tensor(n_s[:], nn[:, 0:512], r_s[:], op=ALU.mult)
                nc.vector.tensor_tensor(n_s[:], n_s[:], nn[:, 512:1024], op=ALU.add)
                nc.scalar.activation(n_s[:], n_s[:], AF.Tanh)
                nc.vector.tensor_tensor(n_s[:], n_s[:], omz[:], op=ALU.mult)
                nc.vector.tensor_tensor(hst[:], n_s[:], zh[:], op=ALU.add)
                tph = tpps2.tile([128, KH + 1, BL], dt.bfloat16, tag="tp2",
                                 name=f"tph_{t}")
                stg = work.tile([BL, 512], dt.bfloat16, tag="stg", name=f"dstg{t}")
                transposes(tph[:, 0:KH, :], hst, stg, hTn[:, 0:4, :], hTn[:, 4:8, :])
                # --- logits, argmax, next embedding, exp-sum for log-softmax ---
                for k in range(KH):
                    nc.tensor.matmul(lp, hTn[:, k, :], outWTs[:, k, :],
                                     start=(k == 0),
                                     stop=(k == KH - 1 and "lgb" not in bias_t))
                if "lgb" in bias_t:
                    nc.tensor.matmul(lp, ones_sb[:], bias_t["lgb"][:],
                                     start=False, stop=True)
                lgt = small.tile([BL, T], dt.float32, tag="lg", name=f"lg{t}")
                lg = lgt[:]
                nc.vector.tensor_copy(lg, lp)
                nc.sync.dma_start(out_r[:, t, :], lg)
                if t < L - 1:
                    mx2 = small.tile([BL, 1], dt.float32, tag="mx2")
                    nc.vector.tensor_reduce(mx2[:], lg, axis=AX.X, op=ALU.max)
                    oh = small.tile([BL, T], dt.bfloat16, tag="oh")
                    nc.vector.tensor_scalar(oh[:], lg, mx2[:], None, op0=ALU.is_equal)
                    ohp = tpps2.tile([128, KH + 1, BL], dt.bfloat16, tag="tp2",
                                     name=f"ohp_{t}")
                    nc.tensor.transpose(ohp[:, 0, :], oh[:], Istk[0:BL, :])
                    ohT = small.tile([128, BL], dt.bfloat16, tag="ohT")
                    nc.vector.tensor_copy(ohT[:], ohp[:, 0, :])
                    for d2 in range(2):
                        nc.tensor.matmul(ep[d2], embbf[:, d2 * 128:(d2 + 1) * 128],
                                         ohT[:], start=True, stop=True)
                    nc.vector.tensor_copy(embT[:, 0, :], ep[0])
                    nc.scalar.copy(embT[:, 1, :], ep[1])
    nc.finalize()
    return nc


def kernel(**inputs):
    import concourse.bass_utils as bass_utils

    tokens = np.asarray(inputs["tokens"])
    w2v = np.asarray(inputs["w2v"], np.float32)
    bn1 = np.asarray(inputs["bn1"], np.float32)
    bn2 = np.asarray(inputs["bn2"], np.float32)
    s1 = float(bn1[0] / np.sqrt(bn1[3] + BN_EPS))
    t1 = float(bn1[1] - bn1[2] * s1)
    s2 = float(bn2[0] / np.sqrt(bn2[3] + BN_EPS))
    t2 = float(bn2[1] - bn2[2] * s2)

    f32 = lambda k: np.asarray(inputs[k], np.float32)
    bft = lambda a: np.ascontiguousarray(np.asarray(a, np.float32).T).astype(BF16)
    enc_bih, enc_bhh = f32("enc_bih"), f32("enc_bhh")
    dec_bih, dec_bhh = f32("dec_bih"), f32("dec_bhh")
    out_W = f32("out_W")
    lgb = (f32("out_b") + t1 * out_W.sum(axis=1))[None, :]
    combb = (f32("comb_b") + t2 / s2)[None, :]

    def opt_bias(row):  # ship only if nonzero
        return None if np.all(row == 0.0) else np.ascontiguousarray(row).astype(BF16)

    biases = {
        "ebhn": opt_bias(enc_bhh[2 * H:][None, :]),
        "dgibrz": opt_bias((dec_bih[:2 * H] + dec_bhh[:2 * H])[None, :]),
        "dbhn": opt_bias(dec_bhh[2 * H:][None, :]),
        "dgibn": opt_bias(dec_bih[2 * H:][None, :]),
        "combb": opt_bias(combb),
        "attnb": opt_bias(f32("attn_b")[None, :]),
        "lgb": opt_bias(lgb),
    }

    # dec_emb renorm (max_norm=1): rows 0..127 for the lookup, row 128 = SOS
    dec_emb = f32("dec_emb")
    nrm = np.linalg.norm(dec_emb, axis=-1, keepdims=True)
    emb_rn = dec_emb * np.minimum(1.0, MAXN2 / (nrm + 1e-7))
    embbf = np.ascontiguousarray(emb_rn[:T]).astype(BF16)
    sos = emb_rn[T]  # (256,)
    sosT = np.zeros((128, 2 * BL), np.float32)
    for k in range(2):
        sosT[:, k * BL:(k + 1) * BL] = sos[k * 128:(k + 1) * 128][:, None]

    istk = np.zeros((128, BL), np.float32)
    istk[np.arange(128), np.arange(128) % BL] = 1.0

    common = {
        "encWhhT": bft(inputs["enc_Whh"]), "decWihT": bft(inputs["dec_Wih"]),
        "decWhhT": bft(inputs["dec_Whh"]), "combWT": bft(inputs["comb_W"]),
        "outWTs": np.ascontiguousarray((s1 * out_W).T).astype(BF16),
        "attnWT": bft(inputs["attn_W"]),
        "embbf": embbf, "sosT": sosT.astype(BF16), "istk": istk.astype(BF16),
    }
    for k, v in biases.items():
        if v is not None:
            common[k] = v

    # host-side encoder input projection, rows = t*128 + hc*64 + b
    enc_Wih = f32("enc_Wih")
    gi_bias = np.concatenate([
        enc_bih[:H] + enc_bhh[:H], enc_bih[H:2 * H] + enc_bhh[H:2 * H],
        enc_bih[2 * H:]])  # (3072,)
    in_maps = []
    for c in range(NC):
        tok = tokens[c * BL:(c + 1) * BL].astype(np.int64)   # (64,25)
        x = w2v[tok]                                         # (64,25,300)
        n = np.linalg.norm(x, axis=-1, keepdims=True)
        x = x * np.minimum(1.0, MAXN1 / (n + 1e-7))
        gi = x.astype(np.float32) @ enc_Wih.T + gi_bias      # (64,25,3072)
        gi = gi.transpose(1, 0, 2).reshape(L, BL, 3, 2, 512)
        gi = gi.transpose(0, 3, 1, 2, 4).reshape(L * 2 * BL, 1536)
        m = dict(common)
        m["gi"] = np.ascontiguousarray(gi).astype(BF16)
        in_maps.append(m)

    nc = build_nc(s2, biases)
    trace = bool(int(os.environ.get("KERNEL_TRACE", "0")))
    res = bass_utils.run_bass_kernel_spmd(nc, in_maps, core_ids=list(range(NC)),
                                          trace=trace)
    if trace and res.exec_time_ns is not None:
        print(f"HW exec time: {res.exec_time_ns} ns", flush=True)
        print("trace:", res.instructions_and_trace[1] if res.instructions_and_trace else None,
              flush=True)
    lg = np.concatenate([res.results[c]["out"] for c in range(NC)], axis=0)
    lg = lg.astype(np.float32)
    mx = lg.max(axis=-1, keepdims=True)
    lse = mx + np.log(np.exp(lg - mx).sum(axis=-1, keepdims=True))
    return lg - lse


if __name__ == "__main__":
    pass
